# revision 25
# baseline (speedup 1.0000x reference)
"""Biaffine NER model (2-layer BiLSTM + highway + biaffine) on 8 Trainium2 cores.

Strategy:
  - Data-parallel over batch: each of the 8 cores handles B_loc=2 of the 16
    batch elements, full model, no collectives.
  - The LSTM recurrences are solved by fixed-point (Jacobi) iteration:
      H^{k+1} = LSTMCell(x_tilde + shift(H^k) @ W_h)
    Each iteration is fully parallel over time; the cell-state recurrence
    c_t = a_t*c_{t-1} + b_t is computed with the hardware tensor_tensor_scan.
  - Iteration schedule per layer: iteration 1 is matmul-free (h_prev == h0
    everywhere, so z0 = h0 @ W_h + b is precomputed on host and enters
    through the activation bias); middle iterations run the recurrent matmul
    in fp8 DoubleRow mode (2 k-blocks per pass at 0.5 cycles/row; weights
    scaled by SW=256 to stay in e4m3's normal range, x_tilde injected through
    a SW-scaled identity, activations descale by 1/SW); the final iteration
    runs in bf16 so fp8 quantization noise is contracted away (~4x/iter).
  - Gate tensors are bf16 (2x/4x DVE modes); the c-scan runs on the idle
    GPSIMD/Pool engine; decay = 1-i via tensor_scalar on DVE.
  - Everything on-chip is "transposed" (feature-major, [128-partition folds,
    (b, t) free]); biases ride as an extra contraction row (ones rail).
"""

import sys

sys.path.insert(0, "/opt/trn_rl_repo")

import ml_dtypes
import numpy as np

import concourse.bass as bass
import concourse.mybir as mybir
import concourse.tile as tile
from concourse.bass_utils import run_bass_kernel_spmd
from concourse.masks import make_identity

F32 = mybir.dt.float32
BF16 = mybir.dt.bfloat16
FP8 = mybir.dt.float8e4
BF16NP = ml_dtypes.bfloat16
FP8NP = ml_dtypes.float8_e4m3
AF = mybir.ActivationFunctionType
ALU = mybir.AluOpType
DR = mybir.MatmulPerfMode.DoubleRow

B, T, D = 16, 256, 768
H, H2 = 400, 800
F, C = 150, 8
NCORES = 8
BL = B // NCORES          # 2 batch elements per core
L = BL * T                # 512 (b, t) rows per core
GP = 512                  # per-gate padded stride (3*GP = 1536, 12 M-tiles)
NM = 12                   # M-tiles of the padded gate dim
KH = [(0, 128), (128, 256), (256, 384), (384, 401)]   # K-tiles of [H+1]
KD = [(k * 128, (k + 1) * 128) for k in range(6)]      # K-tiles of D=768
SW = 128.0                # fp8 scale (must stay < 224, e4m3 max finite)
# per-layer Jacobi schedule: "bias" = matmul-free warm start, "f8" = fp8
# DoubleRow recurrent matmul, "bf16" = full-precision iteration (must be last)
SCHED = ("bias", "f8", "f8", "f8", "bf16")
SCAN_ON_POOL = False

_CACHE = {}


# ------------------------------------------------------------------ host packing

def _pack_gate_cols(w):
    """[K, 3H] -> [K, 3*GP] with each gate's 400 cols padded to 512."""
    k = w.shape[0]
    out = np.zeros((k, 3 * GP), np.float32)
    for g in range(3):
        out[:, g * GP:g * GP + H] = w[:, g * H:(g + 1) * H]
    return np.ascontiguousarray(out)


def _with_bias_row(w, bias):
    """Append one row (the bias, already packed like w's columns) to w."""
    return np.ascontiguousarray(np.concatenate([w, bias[None, :]], 0))


def _fold128(v, nchunk):
    """[n] -> [128, nchunk] column-major fold (unit u -> [u%128, u//128])."""
    out = np.zeros((128, nchunk), np.float32)
    n = len(v)
    for m in range(nchunk):
        seg = v[m * 128:min((m + 1) * 128, n)]
        out[:len(seg), m] = seg
    return out


def _pack_wh8(whp, bias):
    """Gate-padded weights [>=400 rows, M] (+ bias row at k=400) ->
    DoubleRow fp8 layout [128, 2 pairs, 2, M], scaled by SW."""
    m = whp.shape[1]
    wfull = np.zeros((512, m), np.float32)
    wfull[:400] = whp[:400]
    wfull[400] = bias          # bias row at k=400 (chunk 3, partition 16)
    w8 = np.zeros((128, 2, 2, m), np.float32)
    for pair in range(2):
        for i in range(2):
            blk = (pair * 2 + i) * 128
            w8[:, pair, i, :] = wfull[blk:blk + 128] * SW
    return w8.astype(FP8NP)


def _pack_inputs(inputs):
    """Pack weights into the DRAM layouts the program expects (shared by all cores)."""
    f32 = lambda a: np.ascontiguousarray(np.asarray(a, np.float32))
    x = f32(inputs["x"])
    h0 = f32(inputs["h0"])[0]

    packs = {}
    z0all = np.zeros((128, 4, NM), np.float32)   # per-stream h0@Wh+b, gate-folded
    stream_ix = {"0f": 0, "0b": 1, "1f": 2, "1b": 3}

    # layer 0: W [D+H, 3H]
    for nm, wn, bn in (("0f", "W_f0", "b_f0"), ("0b", "W_b0", "b_b0")):
        Wm = f32(inputs[wn]); bias = _pack_gate_cols(f32(inputs[bn])[None, :])[0]
        packs["wx" + nm] = _pack_gate_cols(Wm[:D]).astype(BF16NP)
        whp = _pack_gate_cols(Wm[D:])
        packs["wh" + nm] = _with_bias_row(whp, bias).astype(BF16NP)
        packs["wh8" + nm] = _pack_wh8(whp, bias)
        z0 = h0 @ whp + bias
        for m in range(NM):
            z0all[:, stream_ix[nm], m] = _fold128(z0[m * 128:(m + 1) * 128], 1)[:, 0]
    # layer 1: W [2H+H, 3H]; the input half splits into hf/hb parts (both with
    # zero bias rows -- the bias lives only in wh).
    z = np.zeros((3 * GP,), np.float32)
    for nm, wn, bn in (("1f", "W_f1", "b_f1"), ("1b", "W_b1", "b_b1")):
        Wm = f32(inputs[wn]); bias = _pack_gate_cols(f32(inputs[bn])[None, :])[0]
        packs["wx" + nm + "f"] = _with_bias_row(
            _pack_gate_cols(Wm[:H]), z).astype(BF16NP)
        packs["wx" + nm + "b"] = _with_bias_row(
            _pack_gate_cols(Wm[H:H2]), z).astype(BF16NP)
        whp = _pack_gate_cols(Wm[H2:])
        packs["wh" + nm] = _with_bias_row(whp, bias).astype(BF16NP)
        packs["wh8" + nm] = _pack_wh8(whp, bias)
        z0 = h0 @ whp + bias
        for m in range(NM):
            z0all[:, stream_ix[nm], m] = _fold128(z0[m * 128:(m + 1) * 128], 1)[:, 0]
    packs["z0all"] = z0all

    # highway: W_hw [2H, 2H]; M packed as [f-half pad 512 | b-half pad 512]
    Whw = f32(inputs["W_hw"]); bhw = f32(inputs["b_hw"])

    def _pack_hw_cols(w):
        k = w.shape[0]
        out = np.zeros((k, 2 * GP), np.float32)
        out[:, 0:H] = w[:, 0:H]
        out[:, GP:GP + H] = w[:, H:H2]
        return out

    zh = np.zeros((2 * GP,), np.float32)
    packs["whw8f"] = _pack_wh8(_pack_hw_cols(Whw[:H]), _pack_hw_cols(bhw[None, :])[0])
    packs["whw8b"] = _pack_wh8(_pack_hw_cols(Whw[H:]), zh)

    # projections: Ws/We [2H, F]
    for nm, wn, bn in (("s", "W_s", "b_s"), ("e", "W_e", "b_e")):
        Wm = f32(inputs[wn]); bias = f32(inputs[bn])
        packs["w" + nm + "f"] = _with_bias_row(Wm[:H], bias).astype(BF16NP)
        packs["w" + nm + "b"] = _with_bias_row(
            Wm[H:], np.zeros((F,), np.float32)).astype(BF16NP)

    # biaffine U [F+1, C, F+1] -> [F+1, C*256] (each c padded 151->256)
    U = f32(inputs["U"])
    upk = np.zeros((F + 1, C * 256), np.float32)
    for c in range(C):
        upk[:, c * 256:c * 256 + F + 1] = U[:, c, :]
    packs["upk"] = upk.astype(BF16NP)

    id8 = np.zeros((128, 2, 2, 128), np.float32)
    id8[:, 0, 0, :] = np.eye(128) * SW      # variant A: [SW*I, 0]
    id8[:, 1, 1, :] = np.eye(128) * SW      # variant B: [0, SW*I]
    packs["id8"] = id8.astype(FP8NP)

    h0f = _fold128(h0, 4)
    hti = np.zeros((128, 4, BL, T + 1), np.float32)
    hti[:, :, :, 0] = h0f[:, :, None]          # slot 0 = h0
    hti[16, 3, :, :] = 1.0                     # ones rail for the bias rows
    packs["hti"] = hti.astype(BF16NP)
    packs["hti8"] = hti.astype(FP8NP)
    packs["c0f"] = _fold128(f32(inputs["c0"])[0], 4)

    # per-core x, feature-major [D, L] (time-reversal happens on-chip)
    per_core = []
    for c in range(NCORES):
        sl = x[c * BL:(c + 1) * BL]
        m = dict(packs)
        m["xT"] = np.ascontiguousarray(sl.transpose(2, 0, 1).reshape(D, L)).astype(BF16NP)
        per_core.append(m)
    return per_core


# ------------------------------------------------------------------ program

def _build_program():
    nc = bass.Bass(trn_type="TRN2", target_bir_lowering=False, debug=False)

    dins = {}

    def din(name, shape, dt=BF16):
        dins[name] = nc.dram_tensor(name, list(shape), dt, kind="ExternalInput").ap()
        return dins[name]

    din("xT", (D, L))
    din("wx0f", (D, 3 * GP)); din("wx0b", (D, 3 * GP))
    din("wh0f", (H + 1, 3 * GP)); din("wh0b", (H + 1, 3 * GP))
    for s in ("0f", "0b", "1f", "1b"):
        din("wh8" + s, (128, 2, 2, 3 * GP), dt=FP8)
    for s in ("1f", "1b"):
        din("wx" + s + "f", (H + 1, 3 * GP))
        din("wx" + s + "b", (H + 1, 3 * GP))
        din("wh" + s, (H + 1, 3 * GP))
    din("whw8f", (128, 2, 2, 2 * GP), dt=FP8)
    din("whw8b", (128, 2, 2, 2 * GP), dt=FP8)
    din("wsf", (H + 1, F)); din("wsb", (H + 1, F))
    din("wef", (H + 1, F)); din("web", (H + 1, F))
    din("upk", (F + 1, C * 256))
    din("hti", (128, 4, BL, T + 1)); din("hti8", (128, 4, BL, T + 1), dt=FP8)
    din("c0f", (128, 4), dt=F32); din("z0all", (128, 4, NM), dt=F32)
    din("id8", (128, 2, 2, 128), dt=FP8)
    out_d = nc.dram_tensor("out", [BL, T, T, C], BF16, kind="ExternalOutput").ap()
    dbg = {}
    if DEBUG:
        dbg["xt0b"] = nc.dram_tensor("dbg_xt0b", [128, NM, GP], BF16,
                                     kind="ExternalOutput").ap()
        dbg["ht0f"] = nc.dram_tensor("dbg_ht0f", [128, 4, BL, T + 1], BF16,
                                     kind="ExternalOutput").ap()
        dbg["ht8f"] = nc.dram_tensor("dbg_ht8f", [128, 4, BL, T + 1], FP8,
                                     kind="ExternalOutput").ap()
        dbg["xt1f"] = nc.dram_tensor("dbg_xt1f", [128, NM, GP], BF16,
                                     kind="ExternalOutput").ap()
        dbg["ht1f"] = nc.dram_tensor("dbg_ht1f", [128, 4, BL, T + 1], BF16,
                                     kind="ExternalOutput").ap()
        dbg["outTf"] = nc.dram_tensor("dbg_outTf", [128, 4, BL, T + 1], BF16,
                                      kind="ExternalOutput").ap()
        dbg["s1T"] = nc.dram_tensor("dbg_s1T", [128, 2, L], BF16,
                                    kind="ExternalOutput").ap()

    with tile.TileContext(nc) as tc:
        _body(nc, tc, dins, out_d, dbg)
    _split_multi_waits(nc)
    return nc


def _split_multi_waits(nc, max_waits=1):
    """This container's walrus supports only one embedded sync-wait per
    instruction ("Too many sync wait commands"); hoist extra waits onto
    single-wait NoOps inserted just before, on the same engine queue."""
    n = 0
    for func in nc.m.functions:
        for blk in func.blocks:
            out = []
            for inst in blk.instructions:
                si = inst.sync_info
                if si is not None and si.on_wait and len(si.on_wait) > max_waits:
                    waits = list(si.on_wait)
                    for j, w in enumerate(waits[:-max_waits]):
                        nop = mybir.InstNoOp(name=f"{inst.name}-xw{j}")
                        nop.engine = inst.engine
                        nop.sync_info = mybir.SyncInfo(on_wait=[w], on_update=[])
                        out.append(nop)
                        n += 1
                    inst.sync_info = mybir.SyncInfo(
                        on_wait=waits[-max_waits:], on_update=list(si.on_update))
                out.append(inst)
            blk.instructions = out
    return n


def _load_ktiles(nc, pool, dram, ktiles, cols, tagp):
    tiles = []
    for i, (a, b) in enumerate(ktiles):
        t = pool.tile([b - a, cols], BF16, name=f"{tagp}_{i}", tag=f"{tagp}_{i}")
        nc.sync.dma_start(out=t, in_=dram[a:b, :])
        tiles.append(t)
    return tiles


def _body(nc, tc, dins, out_d, dbg=None):
    # Pool allocation order is the (LIFO) release order, reversed.
    const = tc.alloc_tile_pool(name="const", bufs=1)
    ppool = tc.alloc_tile_pool(name="psum", bufs=2, space="PSUM")
    sepool = tc.alloc_tile_pool(name="se", bufs=1)        # s1/e1 (+ ones rows)
    ht0pool = tc.alloc_tile_pool(name="ht0", bufs=1)      # f/br; reused as blend out
    ssbpool = tc.alloc_tile_pool(name="osb", bufs=2)      # output staging
    latew = tc.alloc_tile_pool(name="latew", bufs=1)      # proj + biaffine weights
    trans = tc.alloc_tile_pool(name="trans", bufs=1)      # gate tiles; till end of E
    ht1pool = tc.alloc_tile_pool(name="ht1", bufs=1)      # f/b/br; released end of E
    ht8pool = tc.alloc_tile_pool(name="ht8", bufs=1)      # fp8 h states; till end E
    xtpool = tc.alloc_tile_pool(name="xtilde", bufs=1)    # x~ slots shared by L0/L1
    wh8pool = tc.alloc_tile_pool(name="wh8", bufs=1)      # fp8 recurrent weights
    ht0tmp = tc.alloc_tile_pool(name="ht0tmp", bufs=1)    # b; released end of C

    ident = const.tile([128, 128], BF16)
    make_identity(nc, ident)
    identS8 = const.tile([128, 2, 2, 128], FP8)
    c0sb = const.tile([128, 4], F32)
    z0sb = const.tile([128, 4, NM], F32)
    ones_c = const.tile([1, BL, T + 1], BF16)
    nc.vector.memset(ones_c, 1.0)

    # recurrence state tensors, allocated and initialized up front on fresh
    # SBUF so their init DMAs carry at most one sync wait each
    ht0 = {}
    ht1 = {}
    ht0["f"] = ht0pool.tile([128, 4, BL, T + 1], BF16, name="ht0f", tag="ht0f")
    ht0["br"] = ht0pool.tile([128, 4, BL, T + 1], BF16, name="ht0br", tag="ht0br")
    ht0["b"] = ht0tmp.tile([128, 4, BL, T + 1], BF16, name="ht0b", tag="ht0b")
    ht0["fr"] = ht0tmp.tile([128, 4, BL, T + 1], BF16, name="ht0fr", tag="ht0fr")
    ht1["f"] = ht1pool.tile([128, 4, BL, T + 1], BF16, name="ht1f", tag="ht1f")
    ht1["b"] = ht1pool.tile([128, 4, BL, T + 1], BF16, name="ht1b", tag="ht1b")
    ht1["br"] = ht1pool.tile([128, 4, BL, T + 1], BF16, name="ht1br", tag="ht1br")
    ht8 = {}
    for si in range(2):
        t_ = ht8pool.tile([128, 4, BL, T + 1], FP8, name=f"ht8_{si}", tag=f"ht8_{si}")
        ht8[si] = t_
    ht8r = {1: ht8pool.tile([128, 4, BL, T + 1], FP8, name="ht8r_1",
                            tag="ht8r_1")}
    s1T = {}
    for nm in ("s", "e"):
        st = sepool.tile([128, 2, L], BF16, name=nm + "1T", tag=nm + "1T")
        s1T[nm] = st

    def psum_tile():
        return ppool.tile([128, 4, GP], F32, name="pz", tag="pz")

    # -------- phase A: layer-0 x_tilde (feature-major) --------
    # DMA order interleaves x k-tiles with weight k-tiles so the first
    # matmuls can start after ~2 tiles instead of after all weight bytes.
    xpool = tc.alloc_tile_pool(name="xt", bufs=1)
    wxpool = tc.alloc_tile_pool(name="wx0", bufs=1)
    wx_sb = {}
    x_sb = {}
    for s, wname in (("f", "wx0f"), ("b", "wx0b")):
        dq = nc.sync if s == "f" else nc.scalar
        xts, wts = [], []
        for i, (a, b) in enumerate(KD):
            if s == "f":
                xt_ = xpool.tile([b - a, L], BF16, name=f"x{s}_{i}", tag=f"x{s}_{i}")
                dq.dma_start(out=xt_, in_=dins["xT"][a:b, :])
            else:
                # time-reverse per batch element by reading the f tile with a
                # negative-stride moving AP
                xt_ = x_sb["f"][i].rearrange(
                    "p (b t) -> p b t", b=BL)[:, :, ::-1]
            wt_ = wxpool.tile([b - a, 3 * GP], BF16, name=f"wx0{s}_{i}",
                              tag=f"wx0{s}_{i}")
            dq.dma_start(out=wt_, in_=dins[wname][a:b, :])
            xts.append(xt_); wts.append(wt_)
        x_sb[s] = xts; wx_sb[s] = wts

    # state inits: issued on the SP queue behind the f-direction loads so the
    # first matmuls aren't delayed; all are needed only once phase B starts
    nc.sync.dma_start(out=c0sb, in_=dins["c0f"])
    nc.sync.dma_start(out=z0sb, in_=dins["z0all"])
    nc.sync.dma_start(out=identS8, in_=dins["id8"])
    for t_ in (ht0["f"], ht0["b"], ht1["f"], ht1["b"]):
        nc.sync.dma_start(out=t_, in_=dins["hti"])
    for si in range(2):
        nc.sync.dma_start(out=ht8[si], in_=dins["hti8"])
    for nm in ("s", "e"):
        nc.sync.dma_start(out=s1T[nm][F - 128:F - 127, 1, :],
                          in_=ones_c.rearrange("p b t -> p (b t)")[:, 0:L])

    xt0 = {}
    for s in ("f", "b"):
        wt, mov = wx_sb[s], x_sb[s]
        store = xtpool.tile([128, NM, GP], BF16, name="xt0" + s, tag="xt" + s)
        for grp in range(3):
            pz = psum_tile()
            for mi in range(4):
                m = grp * 4 + mi
                for k in range(6):
                    nc.tensor.matmul(pz[:, mi, :], wt[k][:, m * 128:(m + 1) * 128],
                                     mov[k], start=(k == 0), stop=(k == 5))
            nc.scalar.copy(store[:, grp * 4:(grp + 1) * 4, :], pz)
        xt0[s] = store
    wxpool.release()
    xpool.release()

    # gate work tiles (bf16), shared across layers via tags
    gtiles = {}
    for si in range(2):
        for nm in ("Dc", "Gt", "O"):
            cols = T if nm == "O" else T + 1
            gtiles[nm, si] = trans.tile([128, 4, BL, cols], BF16,
                                        name=f"{nm}{si}", tag=f"{nm}{si}")
        # scan reset columns: decay 0 / increment c0 at t=0 of every (k, b)
        # window lets one tensor_tensor_scan cover all chunks and batches
        nc.vector.memset(gtiles["Dc", si][:, :, :, 0:1], 0.0)
        for b in range(BL):
            nc.vector.tensor_copy(gtiles["Gt", si][:, :, b, 0], c0sb)

    xt8 = {}
    for si in range(2):
        xt8[si] = xtpool.tile([128, NM, GP], FP8, name=f"xt8_{si}",
                              tag=f"xt8_{si}")

    def jacobi_iteration(si, mode, xs, wh_tiles, w8, ht_bf, h8, z0col, write_fp8,
                         also_fp8=False):
        """One Jacobi iteration for one stream.
        si: stream slot (0/1) for work-tile tags; xs: x_tilde [128, NM, GP];
        wh_tiles: bf16 recurrent weight k-tiles; w8: fp8 DoubleRow weights;
        ht_bf / h8: bf16 / fp8 state tiles; z0col: z0sb[:, stream, :] for
        "bias" mode; write_fp8: write h into h8 (else into ht_bf)."""
        Dc = gtiles["Dc", si]; Gt = gtiles["Gt", si]; O = gtiles["O", si]
        # I and Ct rotate through one shared buffer (tag I{si}): I's last read
        # (the inc multiply) precedes the scan that produces Ct
        I = trans.tile([128, 4, BL, T], BF16, name=f"I{si}", tag=f"I{si}")
        gsl = Gt[:, :, :, 1:T + 1]
        funcs = ((I, AF.Sigmoid), (gsl, AF.Tanh), (O, AF.Sigmoid))
        if mode == "bias":
            for g, (dst, fn) in enumerate(funcs):
                for ch in range(4):
                    m = g * 4 + ch
                    nc.scalar.activation(
                        dst[:, ch], xs[:, m, :].rearrange("p (b t) -> p b t", b=BL),
                        fn, bias=z0col[:, m:m + 1])
        else:
            for g, (dst, fn) in enumerate(funcs):
                pz = psum_tile()
                for mi in range(4):
                    m = g * 4 + mi
                    if mode == "f8":
                        if m < NM - 1:
                            nc.tensor.matmul(pz[:, mi, :], identS8[:, 0],
                                             xt8[si][:, m:m + 2, :],
                                             start=True, stop=False, perf_mode=DR)
                        else:
                            nc.tensor.matmul(pz[:, mi, :], identS8[:, 1],
                                             xt8[si][:, m - 1:m + 1, :],
                                             start=True, stop=False, perf_mode=DR)
                        for pair in range(2):
                            nc.tensor.matmul(
                                pz[:, mi, :],
                                w8[:, pair, :, m * 128:(m + 1) * 128],
                                h8[:, 2 * pair:2 * pair + 2, :, 0:T],
                                start=False, stop=(pair == 1), perf_mode=DR)
                    else:  # bf16
                        nc.tensor.matmul(pz[:, mi, :], ident, xs[:, m, :],
                                         start=True, stop=False)
                        for k in range(4):
                            a, bnd = KH[k]
                            nc.tensor.matmul(pz[:, mi, :],
                                             wh_tiles[k][:, m * 128:(m + 1) * 128],
                                             ht_bf[0:bnd - a, k, :, 0:T],
                                             start=False, stop=(k == 3))
                scale = (1.0 / SW) if mode == "f8" else 1.0
                nc.scalar.activation(
                    dst, pz.rearrange("p m (b t) -> p m b t", b=BL), fn, scale=scale)
        # decay = 1 - i  (DVE, 4x bf16)
        nc.vector.tensor_scalar(out=Dc[:, :, :, 1:T + 1], in0=I, scalar1=-1.0,
                                scalar2=1.0, op0=ALU.mult, op1=ALU.add)
        nc.vector.tensor_mul(gsl, I, gsl)                   # inc = i * g
        # one scan across (chunk, b, t): the reset columns re-seed c0 at the
        # start of every (chunk, b) window
        Ct = trans.tile([128, 4, BL, T + 1], BF16, name=f"Ct{si}", tag=f"I{si}")
        flat = lambda a: a.rearrange("p k b t -> p (k b t)")
        nc.vector.tensor_tensor_scan(
            out=flat(Ct), data0=flat(Dc), data1=flat(Gt), initial=0.0,
            op0=ALU.mult, op1=ALU.add)
        nc.scalar.activation(Ct, Ct, AF.Tanh)
        tgt = h8 if write_fp8 else ht_bf
        nc.vector.tensor_mul(tgt[:, 0:3, :, 1:T + 1], Ct[:, 0:3, :, 1:T + 1],
                             O[:, 0:3])
        nc.vector.tensor_mul(tgt[0:16, 3, :, 1:T + 1], Ct[0:16, 3, :, 1:T + 1],
                             O[0:16, 3])
        if also_fp8:
            nc.vector.tensor_mul(h8[:, 0:3, :, 1:T + 1], Ct[:, 0:3, :, 1:T + 1],
                                 O[:, 0:3])
            nc.vector.tensor_mul(h8[0:16, 3, :, 1:T + 1],
                                 Ct[0:16, 3, :, 1:T + 1], O[0:16, 3])

    def run_layer(layer, streams):
        """streams: list of (si, stream_key, xs, ht_bf, h8, z0_index)."""
        wh8sb = {}
        for si, sk, xs, ht_bf, h8, z0i in streams:
            t_ = wh8pool.tile([128, 2, 2, 3 * GP], FP8, name="wh8" + sk,
                              tag=f"wh8_{si}")
            nc.scalar.dma_start(out=t_, in_=dins["wh8" + sk])
            wh8sb[sk] = t_
        for si, sk, xs, ht_bf, h8, z0i in streams:
            nc.vector.tensor_copy(xt8[si][:, 0:NM, :], xs)
        whpool = tc.alloc_tile_pool(name=f"wh{layer}", bufs=1)
        whl = {}
        for it, mode in enumerate(SCHED):
            if mode == "bf16" and not whl:
                # bf16 weights load late so their DMA bytes don't crowd the
                # startup window; they land well before the final iteration
                for si, sk, xs, ht_bf, h8, z0i in streams:
                    whl[sk] = _load_ktiles(nc, whpool, dins["wh" + sk], KH,
                                           3 * GP, f"wh_{si}")
            nxt = SCHED[it + 1] if it + 1 < len(SCHED) else None
            for si, sk, xs, ht_bf, h8, z0i in streams:
                jacobi_iteration(si, mode, xs, whl.get(sk), wh8sb[sk], ht_bf, h8,
                                 z0sb[:, z0i, :], write_fp8=(nxt == "f8"),
                                 also_fp8=(nxt is None and layer == 1))
        whpool.release()

    # -------- phase B: layer-0 recurrences --------
    run_layer(0, [(0, "0f", xt0["f"], ht0["f"], ht8[0], 0),
                  (1, "0b", xt0["b"], ht0["b"], ht8[1], 1)])

    # reversed-time copies (the ones rail at [16, 3] copies over too)
    nc.vector.tensor_copy(ht0["fr"][:, :, :, 1:T + 1], ht0["f"][:, :, :, T:0:-1])
    nc.vector.tensor_copy(ht0["br"][:, :, :, 1:T + 1], ht0["b"][:, :, :, T:0:-1])

    # -------- phase C: layer-1 x_tilde --------
    xt1 = {}
    for s, (hfmov, hbmov) in (("f", (ht0["f"], ht0["br"])), ("b", (ht0["fr"], ht0["b"]))):
        wpool = tc.alloc_tile_pool(name="wx1" + s, bufs=1)
        wtf = _load_ktiles(nc, wpool, dins["wx" + "1" + s[0] if False else "wx1" + s + "f"], KH, 3 * GP, "wx1f_t")
        wtb = _load_ktiles(nc, wpool, dins["wx1" + s + "b"], KH, 3 * GP, "wx1b_t")
        store = xtpool.tile([128, NM, GP], BF16, name="xt1" + s, tag="xt" + s)
        pairs = [(wtf[k], hfmov, k) for k in range(4)] + [(wtb[k], hbmov, k) for k in range(4)]
        for grp in range(3):
            pz = psum_tile()
            for mi in range(4):
                m = grp * 4 + mi
                for pi, (wt, mov, k) in enumerate(pairs):
                    a, bnd = KH[k]
                    nc.tensor.matmul(pz[:, mi, :], wt[:, m * 128:(m + 1) * 128],
                                     mov[0:bnd - a, k, :, 1:T + 1],
                                     start=(pi == 0), stop=(pi == 7))
            nc.scalar.copy(store[:, grp * 4:(grp + 1) * 4, :], pz)
        xt1[s] = store
        wpool.release()
    ht0tmp.release()

    if dbg:
        for key, tsrc in (("xt0b", xt0["b"]), ("ht0f", ht0["f"]),
                          ("ht8f", ht8[0]), ("xt1f", xt1["f"])):
            nc.sync.dma_start(out=dbg[key], in_=tsrc)

    # -------- phase D: layer-1 recurrences --------
    run_layer(1, [(0, "1f", xt1["f"], ht1["f"], ht8[0], 2),
                  (1, "1b", xt1["b"], ht1["b"], ht8[1], 3)])
    nc.vector.tensor_copy(ht1["br"][:, :, :, 1:T + 1], ht1["b"][:, :, :, T:0:-1])
    nc.vector.tensor_copy(ht8r[1][:, :, :, 1:T + 1], ht8[1][:, :, :, T:0:-1])
    wh8pool.release()
    xtpool.release()

    if dbg:
        nc.sync.dma_start(out=dbg["ht1f"], in_=ht1["f"])

    # late-phase weights into the up-front pool (DMAs run during D)
    projw = {}
    for nm in ("s", "e"):
        projw[nm] = (_load_ktiles(nc, latew, dins["w" + nm + "f"], KH, F,
                                  "w" + nm + "f"),
                     _load_ktiles(nc, latew, dins["w" + nm + "b"], KH, F,
                                  "w" + nm + "b"))
    KU = [(0, 128), (128, F + 1)]
    ut = _load_ktiles(nc, latew, dins["upk"], KU, C * 256, "upk")
    whw8 = {}
    for hh in ("f", "b"):
        t_ = latew.tile([128, 2, 2, 2 * GP], FP8, name="whw8" + hh,
                        tag="whw8" + hh)
        nc.sync.dma_start(out=t_, in_=dins["whw8" + hh])
        whw8[hh] = t_

    # -------- phase E: highway gate + blend (in place over ht0 f/br slots) ----
    outT = {}
    pairs = [(whw8["f"], ht8[0], p) for p in range(2)] + \
            [(whw8["b"], ht8r[1], p) for p in range(2)]
    for half, (h1, h0) in (("f", (ht1["f"], ht0["f"])), ("b", (ht1["br"], ht0["br"]))):
        pz = psum_tile()
        for mi in range(4):
            m = (0 if half == "f" else 4) + mi
            for pi, (wt, mov, p) in enumerate(pairs):
                nc.tensor.matmul(pz[:, mi, :],
                                 wt[:, p, :, m * 128:(m + 1) * 128],
                                 mov[:, 2 * p:2 * p + 2, :, 1:T + 1],
                                 start=(pi == 0), stop=(pi == 3), perf_mode=DR)
        gate = trans.tile([128, 4, BL, T], BF16, name="hwgate",
                          tag=f"I{0 if half == 'f' else 1}")
        nc.scalar.activation(gate, pz.rearrange("p m (b t) -> p m b t", b=BL),
                             AF.Sigmoid, scale=1.0 / SW)
        tmp = gtiles["Gt", 0 if half == "f" else 1][:, :, :, 1:T + 1]
        hsl = h0[:, :, :, 1:T + 1]
        nc.vector.tensor_sub(tmp, h1[:, :, :, 1:T + 1], hsl)
        nc.vector.tensor_mul(tmp, gate, tmp)
        # the final write skips partition 16 of chunk 3 so the ones rail from
        # the init image survives for the projection bias rows
        nc.vector.tensor_add(hsl[:, 0:3], hsl[:, 0:3], tmp[:, 0:3])
        nc.vector.tensor_add(hsl[0:16, 3], hsl[0:16, 3], tmp[0:16, 3])
        outT[half] = h0
    ht8pool.release()
    ht1pool.release()
    trans.release()

    if dbg:
        nc.sync.dma_start(out=dbg["outTf"], in_=outT["f"])

    # -------- phase F: s/e projections --------
    for nm in ("s", "e"):
        wf, wb = projw[nm]
        st = s1T[nm]
        prs = [(wf[k], outT["f"], k) for k in range(4)] + [(wb[k], outT["b"], k) for k in range(4)]
        pz = psum_tile()
        for mi, (ma, mb) in enumerate(((0, 128), (128, F))):
            for pi, (wt, mov, k) in enumerate(prs):
                a, bnd = KH[k]
                nc.tensor.matmul(pz[0:mb - ma, mi, :], wt[:, ma:mb],
                                 mov[0:bnd - a, k, :, 1:T + 1],
                                 start=(pi == 0), stop=(pi == 7))
        nc.scalar.copy(st[:, 0, :], pz[:, 0, :])
        nc.scalar.copy(st[0:F - 128, 1, :], pz[0:F - 128, 1, :])

    if dbg:
        nc.sync.dma_start(out=dbg["s1T"], in_=s1T["s"])

    # -------- phase G: biaffine part 1: tmp[(c,j), (b,t)] --------
    smov = [s1T["s"][:, 0, :], s1T["s"][0:F + 1 - 128, 1, :]]
    tmpT = sepool.tile([128, 16, GP], BF16, name="tmpT", tag="tmpT")
    for grp in range(4):
        pz = psum_tile()
        for mi in range(4):
            m = grp * 4 + mi
            for k in range(2):
                nc.tensor.matmul(pz[:, mi, :], ut[k][:, m * 128:(m + 1) * 128],
                                 smov[k], start=(k == 0), stop=(k == 1))
        nc.scalar.copy(tmpT[:, grp * 4:(grp + 1) * 4, :], pz)
    latew.release()

    # -------- phase H: biaffine part 2 + output assembly --------
    emov0 = s1T["e"][:, 0, :].rearrange("p (b t) -> p b t", b=BL)
    emov1 = s1T["e"][0:F + 1 - 128, 1, :].rearrange("p (b t) -> p b t", b=BL)
    for b in range(BL):
        for xt_i in range(2):
            osb = ssbpool.tile([128, T, C], BF16, name="osb", tag="osb")
            pz = psum_tile()
            for c in range(C):
                xsl = slice(b * T + xt_i * 128, b * T + xt_i * 128 + 128)
                po = pz[:, c // 2, (c % 2) * T:(c % 2) * T + T]
                nc.tensor.matmul(po, tmpT[:, 2 * c, xsl], emov0[:, b, :],
                                 start=True, stop=False)
                nc.tensor.matmul(po, tmpT[0:F + 1 - 128, 2 * c + 1, xsl],
                                 emov1[:, b, :], start=False, stop=True)
            # split psum evacuation across ACT/DVE/Pool so it pipelines
            for c in range(C):
                src = pz[:, c // 2, (c % 2) * T:(c % 2) * T + T]
                if c % 2 == 0:
                    nc.scalar.copy(osb[:, :, c], src)
                else:
                    nc.vector.tensor_copy(osb[:, :, c], src)
            dq = nc.sync if (b * 2 + xt_i) % 2 == 0 else nc.scalar
            dq.dma_start(out=out_d[b, xt_i * 128:(xt_i + 1) * 128, :, :], in_=osb)
    ssbpool.release()
    ht0pool.release()
    sepool.release()
    ppool.release()
    const.release()


# ------------------------------------------------------------------ entry point

DEBUG = False
TRACE = False          # set True (from test harnesses) to capture an NTFF profile
LAST_RESULT = None     # BassKernelResults of the most recent run


def kernel(**inputs) -> np.ndarray:
    global LAST_RESULT
    if "nc" not in _CACHE:
        _CACHE["nc"] = _build_program()
    nc = _CACHE["nc"]
    in_maps = _pack_inputs(inputs)
    try:
        res = run_bass_kernel_spmd(nc, in_maps, core_ids=list(range(NCORES)),
                                   trace=TRACE)
    except ModuleNotFoundError:
        res = run_bass_kernel_spmd(nc, in_maps, core_ids=list(range(NCORES)))
    LAST_RESULT = res
    out = np.concatenate(
        [np.asarray(res.results[c]["out"]).astype(np.float32)
         for c in range(NCORES)], axis=0)
    return np.ascontiguousarray(out)


if __name__ == "__main__":
    raise SystemExit("use test.py")
